# revision 82
# baseline (speedup 1.0000x reference)
"""Causal self-attention (B=4, T=2048, C=1024, H=16) on 8 trn2 NeuronCores.

Sharding: core c -> (batch b = c//2, head-group g = c%2). Each core owns
heads 8g..8g+7 (feature dims 512g..512g+512) of its batch: it projects
q/k/v only for those 512 dims (no duplicated K/V work across cores),
runs attention for its 8 heads over the full causal sequence, and emits
a partial output projection; the host sums the two head-group partials
per batch and adds the output bias.

Per-core device pipeline (bf16 matmuls, fp32 PSUM accumulation):
  - Projections per head-pair hp (two heads share the 128-partition d):
    kT/qT in transposed [d, t] layout, v natural [t, d] with a ones
    column per head (AV matmul then also yields the softmax denom Z).
  - Attention per head-pair, query blocks J of 512 (natural order),
    key tiles of 128 with 128-granular causal trimming: S^T = K Q^T via
    row-packed tile_position matmuls (two heads concurrent), exp on the
    scalar engine (logits O(6), no max subtraction), diagonal tiles get
    a single 128x128 triangular multiplicative mask, AV accumulated
    over key tiles in PSUM.
  - Software pipelining: projection matmuls of head-pair hp+1 are
    interleaved into the attention kt-loop of hp so the tensor engine
    never waits on the scalar engine's exp stream.
  - Deferred normalization: 1/Z broadcast across partitions via K=1
    matmuls, applied to yT; output projection accumulates the 4 d-chunks
    and DMAs straight from PSUM.
"""

import numpy as np
import ml_dtypes
from contextlib import ExitStack

import concourse.bass as bass
import concourse.bacc as bacc
import concourse.mybir as mybir
import concourse.tile as tile
from concourse import bass_utils

B, T, C, H = 4, 2048, 1024, 16
HD = C // H            # 64
NCORES = 8
CG = C // 2            # 512 feature dims per core (8 heads)
NHP = CG // 128        # 4 head-pairs per core
NCH = C // 128         # 8 contraction chunks over C
NJ = T // 512          # 4 query blocks
SCALE = 1.0 / float(np.sqrt(HD))

bf16 = mybir.dt.bfloat16
f32 = mybir.dt.float32
AF = mybir.ActivationFunctionType

_compiled = {}
last_result = None  # BassKernelResults of the most recent run (for test harness)


def _build():
    nc = bacc.Bacc("TRN2", target_bir_lowering=False, debug=False,
                   num_devices=NCORES)

    # host pre-shuffles inputs to partition-major contiguous layouts so
    # every DMA moves 2-8KB per-partition lines; wq/wk are head-pair-major
    # and wv head-half-major so the first-needed slices are small DMAs
    xT_d = nc.dram_tensor("xTs", [128, NJ, NCH, 512], bf16,
                          kind="ExternalInput")
    wqT_d = nc.dram_tensor("wqs", [128, NHP, NCH, 128], bf16,
                           kind="ExternalInput")
    wkT_d = nc.dram_tensor("wks", [128, NHP, NCH, 128], bf16,
                           kind="ExternalInput")
    wvT_d = nc.dram_tensor("wvs", [128, 2, NCH, 256], bf16,
                           kind="ExternalInput")
    wpT_d = nc.dram_tensor("wps", [128, NHP, C], bf16, kind="ExternalInput")
    bq_d = nc.dram_tensor("bq2", [128, NHP], f32, kind="ExternalInput")
    misc_d = nc.dram_tensor("misc", [128, 384], bf16, kind="ExternalInput")
    out_d = nc.dram_tensor("out", [T, C], bf16, kind="ExternalOutput")

    xT_v = xT_d.ap()
    wq_v = wqT_d.ap()
    wk_v = wkT_d.ap()
    wv_v = wvT_d.ap()
    wp_v = wpT_d.ap()
    out_v = out_d.ap().rearrange("(a p) c -> p a c", p=128)

    with tile.TileContext(nc) as tc, ExitStack() as ctx:
        persist = ctx.enter_context(tc.tile_pool(name="persist", bufs=1))
        pp = ctx.enter_context(tc.tile_pool(name="pp", bufs=1, space="PSUM"))
        spool = ctx.enter_context(
            tc.tile_pool(name="spool", bufs=2, space="PSUM"))
        opool = ctx.enter_context(
            tc.tile_pool(name="opool", bufs=1, space="PSUM"))
        tpool = ctx.enter_context(
            tc.tile_pool(name="tpool", bufs=1, space="PSUM"))
        p2pool = ctx.enter_context(tc.tile_pool(name="p2pool", bufs=4))
        outp = ctx.enter_context(tc.tile_pool(name="outp", bufs=4))
        yfpool = ctx.enter_context(tc.tile_pool(name="yfpool", bufs=2))

        xT_sb = persist.tile([128, NCH, T], bf16)
        wq_sb = persist.tile([128, NHP, NCH, 128], bf16)
        wk_sb = persist.tile([128, NHP, NCH, 128], bf16)
        wv_sb = persist.tile([128, 2, NCH, 256], bf16)
        wp_sb = persist.tile([128, NHP, C], bf16)
        kT_sb = persist.tile([128, NHP, T], bf16)
        qT_sb = persist.tile([128, NHP, T], bf16)
        v_sb = persist.tile([128, 16, 8, HD + 1], bf16)
        yT_sb = persist.tile([128, NHP, T], bf16)
        bq_sb = persist.tile([128, NHP], f32)
        misc_sb = persist.tile([128, 384], bf16)
        mask_ap = misc_sb[:, 0:256].rearrange("p (h q) -> p h q", q=128)
        ident_ap = misc_sb[:, 256:384]

        nc.vector.memset(v_sb[:, :, :, HD:HD + 1], 1.0)

        # input DMAs: the cost model serializes DMAs on one global device at
        # ~360 GB/s, so order IS priority: the tb0 projection set first,
        # then per-need (mask at first exp, bq at first Q bias, identity at
        # the first transpose).
        nc.sync.dma_start(wk_sb[:, 0], wk_v[:, 0])
        nc.sync.dma_start(xT_sb[:, :, 0:512], xT_v[:, 0])
        nc.scalar.dma_start(wq_sb[:, 0], wq_v[:, 0])
        nc.scalar.dma_start(bq_sb[:], bq_d.ap())
        nc.sync.dma_start(misc_sb[:], misc_d.ap())
        nc.scalar.dma_start(wv_sb[:, 0], wv_v[:, 0])
        nc.sync.dma_start(xT_sb[:, :, 512:1024], xT_v[:, 1])
        nc.sync.dma_start(xT_sb[:, :, 1024:1536], xT_v[:, 2])
        nc.scalar.dma_start(wk_sb[:, 1:4], wk_v[:, 1:4])
        nc.sync.dma_start(xT_sb[:, :, 1536:2048], xT_v[:, 3])
        nc.scalar.dma_start(wq_sb[:, 1:4], wq_v[:, 1:4])
        nc.scalar.dma_start(wp_sb[:], wp_v)
        nc.scalar.dma_start(wv_sb[:, 1], wv_v[:, 1])

        # ---------------- projection emitters (pipelined as work items) ----
        def proj_kq(w_sb, b_sb, dst_sb, hp, tb):
            """One 512-col t-block of the kT/qT projection for head-pair hp.

            b_sb None: bias skipped (k-bias is softmax-invariant: it adds a
            per-query constant q_i . bk to every logit of query i).
            """
            ps = pp.tile([128, 512], f32, tag="pp")
            ts = slice(512 * tb, 512 * tb + 512)
            for c in range(NCH):
                nc.tensor.matmul(
                    ps[:], w_sb[:, hp, c, :], xT_sb[:, c, ts],
                    start=(c == 0), stop=(c == NCH - 1))
            if b_sb is None:
                nc.vector.tensor_copy(dst_sb[:, hp, ts], ps[:])
            else:
                nc.vector.tensor_scalar_add(dst_sb[:, hp, ts], ps[:],
                                            b_sb[:, hp:hp + 1])

        def proj_v(half, r):
            """V rows [128r, 128r+128) for head-half `half` (v-bias folded
            into the host-side output bias: (y+Z*bv)/Z@Wp^T = y/Z@Wp^T+bv@Wp^T)."""
            ps = pp.tile([128, 512], f32, tag="pp")
            for c in range(NCH):
                nc.tensor.matmul(
                    ps[:, 0:256], xT_sb[:, c, 128 * r:128 * r + 128],
                    wv_sb[:, half, c, :], start=(c == 0), stop=(c == NCH - 1))
            nc.vector.tensor_copy(
                v_sb[:, r, 4 * half:4 * half + 4, 0:HD],
                ps[:, 0:256].rearrange("p (h e) -> p h e", e=HD))

        def outproj_j(J):
            """Output projection for t-tiles of query block J (all heads)."""
            for tt in range(4 * J, 4 * J + 4):
                ot = outp.tile([128, 1024], bf16, tag="ot")
                for ch in range(2):
                    ps = pp.tile([128, 512], f32, tag="pp")
                    for d in range(NHP):
                        nc.tensor.matmul(
                            ps[:], yT_sb[:, d, 128 * tt:128 * tt + 128],
                            wp_sb[:, d, 512 * ch:512 * ch + 512],
                            start=(d == 0), stop=(d == NHP - 1))
                    if ch == 0:
                        nc.scalar.copy(ot[:, 0:512], ps[:])
                    else:
                        nc.vector.tensor_copy(ot[:, 512:1024], ps[:])
                nc.sync.dma_start(out_v[:, tt, :], ot[:])

        # ---------------- prologue (tb0 only; rest paced by DMA arrival) --
        proj_kq(wk_sb, None, kT_sb, 0, 0)
        proj_kq(wq_sb, bq_sb, qT_sb, 0, 0)
        proj_v(0, 0)
        proj_v(0, 1)

        # ---------------- attention, pipelined with next projections ------
        # post-processing of a finished J block (normalize in the natural
        # [q, d] domain via a broadcast 1/Z multiply, transpose back to
        # [d, q], stash yT) is deferred into the next block's early steps
        # so the tensor engine never waits on it.
        post_v = [None]   # vector part: reciprocal + normalize
        post_t = [None]   # tensor part: transposes + yT copies (+ outproj)

        def make_post(hp, J, oF):
            qs = slice(512 * J, 512 * J + 512)
            # per-head [128, 4 slots, 65] view of that head's PSUM bank
            ovz = oF[:, :, 0:4 * (HD + 1)].rearrange("p h (s e) -> p h s e",
                                                     e=HD + 1)

            def run_v():
                zrn = yfpool.tile([128, 2, 4], f32, tag="zrn")
                nc.vector.reciprocal_approx_fast(
                    zrn[:], ovz[:, :, :, HD:HD + 1].rearrange(
                        "p h s e -> p h (s e)"))
                yf = yfpool.tile([128, 2, 4, HD], bf16, tag="yf")
                for h in range(2):
                    a_ap = ovz[:, h, :, 0:HD]
                    z1 = zrn[:, h, :].rearrange("p (s e) -> p s e", e=1)
                    _, z_ap = bass.broadcast_tensor_aps(a_ap, z1)
                    nc.vector.tensor_mul(yf[:, h], a_ap, z_ap)
                post_t[0] = lambda: run_t(yf)

            def run_t(yf):
                # transposes land head A at partitions 0-63 and head B at
                # 64-127 (column tile position) so one aligned copy suffices
                tps = tpool.tile([128, 4, 128], bf16, tag="tps")
                for h in range(2):
                    for s in range(4):
                        nc.tensor.matmul(tps[64 * h:64 * h + 64, s, :],
                                         yf[:, h, s, :], ident_ap,
                                         is_transpose=True,
                                         tile_position=(0, 64 * h))
                nc.vector.tensor_copy(
                    yT_sb[:, hp, qs].rearrange("p (s q) -> p s q", q=128),
                    tps[:])
                if hp == NHP - 1:
                    outproj_j(J)

            return run_v

        for hp in range(NHP):
            steps = sum(4 * J + 4 for J in range(NJ))  # 40
            sched = {}

            def put(s, item):
                sched.setdefault(s, []).append(item)

            if hp == 0:
                # own K0/Q0 tb1-3 paced to xT t-block DMA arrival (placing
                # them too early FIFO-blocks the exp stream behind the DMA);
                # V row r must precede AV(kt=r) of block J=r//4 (FIFO).
                put(2, ("kq", wk_sb, None, kT_sb, 0, 1))
                put(3, ("kq", wq_sb, bq_sb, qT_sb, 0, 1))
                put(10, ("kq", wk_sb, None, kT_sb, 0, 2))
                put(11, ("kq", wq_sb, bq_sb, qT_sb, 0, 2))
                put(21, ("kq", wk_sb, None, kT_sb, 0, 3))
                put(22, ("kq", wq_sb, bq_sb, qT_sb, 0, 3))
                for r, s in zip(range(2, 8), (1, 3, 5, 7, 9, 11)):
                    put(s, ("v", 0, r))
                for i, r in enumerate(range(8, 16)):
                    put(13 + i * 2, ("v", 0, r))
                kqsteps = [14, 16, 18, 20, 24, 26, 28, 30]
            else:
                kqsteps = [2, 6, 10, 14, 18, 22, 26, 30]
                if hp == 1:
                    for i, r in enumerate(range(16)):
                        put(1 + 2 * i, ("v", 1, r))
            if hp < NHP - 1:
                for tb in range(4):
                    put(kqsteps[tb], ("kq", wk_sb, None, kT_sb, hp + 1, tb))
                for tb in range(4):
                    put(kqsteps[4 + tb],
                        ("kq", wq_sb, bq_sb, qT_sb, hp + 1, tb))
            step = 0

            # last head-pair: big J blocks first so the final serial tail
            # (post chain + output projection of the last J) is the smallest
            jorder = range(NJ - 1, -1, -1) if hp == NHP - 1 else range(NJ)
            for J in jorder:
                # one PSUM bank per head: a 65-wide slot must not cross the
                # 2KB bank boundary
                oF = opool.tile([128, 2, 512], f32, tag="oF")

                def emit_av(item, hp=hp, J=J, oF=oF):
                    kt, pp2, i0 = item
                    for h in range(2):
                        for s in range(i0 // 128, 4):
                            c0 = 512 * h + 128 * s
                            # start only on the first matmul into each
                            # head-bank: start marks the whole 2KB bank
                            # pending-zero, so later slots' first writes
                            # overwrite (and then accumulate) correctly
                            nc.tensor.matmul(
                                oF[:, h, 65 * s:65 * s + 65],
                                pp2[:, c0:c0 + 128],
                                v_sb[:, kt, 2 * hp + h, :],
                                start=(kt == 0 and s == 0),
                                stop=(kt - 4 * J == s),
                                skip_group_check=True)

                pend = []
                jstep = 0
                for kt in range(4 * J + 4):
                    ks = slice(128 * kt, 128 * kt + 128)
                    i0 = 128 * (kt - 4 * J) if kt >= 4 * J else 0
                    s2 = spool.tile([128, 1024], f32, tag="s2")
                    nc.tensor.matmul(
                        s2[:, i0:512], kT_sb[0:64, hp, ks],
                        qT_sb[0:64, hp, 512 * J + i0:512 * J + 512],
                        tile_position=(0, 0))
                    nc.tensor.matmul(
                        s2[:, 512 + i0:1024], kT_sb[64:128, hp, ks],
                        qT_sb[64:128, hp, 512 * J + i0:512 * J + 512],
                        tile_position=(64, 0))
                    p2 = p2pool.tile([128, 1024], bf16, tag="p2")
                    if i0 == 0:
                        nc.scalar.activation(p2[:], s2[:], AF.Exp, scale=SCALE)
                    else:
                        s2v = s2[:].rearrange("p (h q) -> p h q", q=512)
                        p2v = p2[:].rearrange("p (h q) -> p h q", q=512)
                        nc.scalar.activation(p2v[:, :, i0:512],
                                             s2v[:, :, i0:512],
                                             AF.Exp, scale=SCALE)
                    if kt >= 4 * J:  # diagonal tile: 128x128 triangular mask
                        p2v = p2[:].rearrange("p (h q) -> p h q", q=512)
                        nc.vector.tensor_mul(p2v[:, :, i0:i0 + 128],
                                             p2v[:, :, i0:i0 + 128],
                                             mask_ap)
                    if jstep == 0 and post_v[0] is not None:
                        post_v[0]()
                        post_v[0] = None
                    if len(pend) >= 2:
                        emit_av(pend.pop(0))
                    if jstep == 2 and post_t[0] is not None:
                        post_t[0]()
                        post_t[0] = None
                    pend.append((kt, p2, i0))
                    for w in sched.get(step, ()):
                        if w[0] == "kq":
                            proj_kq(*w[1:])
                        else:
                            proj_v(w[1], w[2])
                    step += 1
                    jstep += 1
                for item in pend:
                    emit_av(item)
                post_v[0] = make_post(hp, J, oF)

        post_v[0]()
        post_t[0]()

    nc.compile()
    return nc


def prep_in_maps(x, Wq, bq, Wk, bk, Wv, bv, Wp, bp):
    x = np.asarray(x, dtype=np.float32)
    Wq = np.asarray(Wq, dtype=np.float32)
    Wk = np.asarray(Wk, dtype=np.float32)
    Wv = np.asarray(Wv, dtype=np.float32)
    Wp = np.asarray(Wp, dtype=np.float32)
    bq = np.asarray(bq, dtype=np.float32)
    bk = np.asarray(bk, dtype=np.float32)
    bv = np.asarray(bv, dtype=np.float32)

    bf = ml_dtypes.bfloat16
    kk = np.arange(128)[:, None]
    jj = np.arange(128)[None, :]
    tri = (kk <= jj).astype(bf)
    misc = np.ascontiguousarray(
        np.concatenate([tri, tri, np.eye(128, dtype=bf)], axis=1))

    def shuf_w(wT, nsl, width):
        # [C_in, nsl*width] -> [128 p, nsl, C_in//128, width]
        n = wT.shape[0] // 128
        return np.ascontiguousarray(
            wT.reshape(n, 128, nsl, width).transpose(1, 2, 0, 3)).astype(bf)

    # x[b].T is [C, T]; -> [128 p, 4 tb, 8 c, 512 t]
    xTs = [np.ascontiguousarray(
        x[b].T.reshape(NCH, 128, NJ, 512).transpose(1, 2, 0, 3)).astype(bf)
        for b in range(B)]
    gslices = [slice(0, CG), slice(CG, C)]
    in_maps = []
    for core in range(NCORES):
        b, g = core // 2, core % 2
        gs = gslices[g]
        in_maps.append({
            "xTs": xTs[b],
            "wqs": shuf_w(Wq[gs, :].T, NHP, 128),
            "wks": shuf_w(Wk[gs, :].T, NHP, 128),
            "wvs": shuf_w(Wv[gs, :].T, 2, 256),
            "wps": shuf_w(Wp[:, gs].T, 1, C).reshape(128, NHP, C),
            "bq2": np.ascontiguousarray(bq[gs].reshape(NHP, 128).T),
            "misc": misc,
        })
    return in_maps


def kernel(x, Wq, bq, Wk, bk, Wv, bv, Wp, bp, **_ignored):
    global last_result
    bp = np.asarray(bp, dtype=np.float32)
    in_maps = prep_in_maps(x, Wq, bq, Wk, bk, Wv, bv, Wp, bp)

    if "nc" not in _compiled:
        _compiled["nc"] = _build()
    nc = _compiled["nc"]

    last_result = bass_utils.run_bass_kernel_spmd(
        nc, in_maps, core_ids=list(range(NCORES)))

    # v-bias folded here: y includes v without bias; (y/Z + bv) @ Wp^T + bp
    bp_eff = bp + np.asarray(bv, dtype=np.float32) @ np.asarray(
        Wp, dtype=np.float32).T
    out = np.empty((B, T, C), dtype=np.float32)
    for b in range(B):
        out[b] = np.asarray(last_result.results[2 * b]["out"],
                            dtype=np.float32)
        out[b] += np.asarray(last_result.results[2 * b + 1]["out"],
                             dtype=np.float32)
    out += bp_eff[None, None, :]
    return out
